# revision 10
# baseline (speedup 1.0000x reference)
"""3-layer GAT (PyG GATConv semantics) on 8 Trainium2 NeuronCores — v4.

Strategy (dst-sharded, per-tile SWDGE gathers, lean compute):
- Nodes assigned to 160 degree-balanced blocks of <=128 dst slots; 20/core.
- Edge phase per block: tile 0 carries the 128 added self-loops and is filled by
  an SBUF copy of the block's own dense output (no gather); tiles 1..T carry
  non-self edges delivered by ONE dma_gather (SWDGE) of [h | alpha_src] rows.
  Per-block index streams are padded to a uniform E_max with row-0 dummies and
  -1 beyond, so the Q7 descriptor loop does exactly E_max rows per block.
- Layer 1's table is computed redundantly by every core from the full x (PE is
  mostly idle), eliminating the serial layer-1 AllGather from the prologue. A
  second 20-block pass over the core's own x shard fills denseT/adl (SPMD
  program is shared, so local-block extraction must use program-uniform
  indices). Layers 2/3 tables are AllGathered in per-quarter chunks fired from
  mid-edge hooks of the previous layer, shrinking inter-layer bubbles.
- The walrus in this toolchain accepts only ONE sync wait per instruction;
  BassOneWait splits Tile-generated multi-waits at serialization.
"""
import numpy as np
from contextlib import ExitStack
import heapq

import orjson
import concourse.bass as bass
import concourse.tile as tile
from concourse import mybir
from concourse.bass_utils import run_bass_kernel_spmd
from concourse.library_config import mlp

# problem constants (fixed by the harness's setup_inputs)
N_NODES = 20000
N_EDGES = 320000
IN_DIM = 128
HID = 64
HEADS = 4
HC = HEADS * HID          # 256
ROWW = HC + HEADS         # 260 = [h | alpha_src] (useful part)
ROWP = HC                 # table row: h only (512B, dma_gather 256B-aligned); per-edge
                          # alpha_src is recomputed on DVE from the gathered h
WAUG = HC + 2 * HEADS     # 264 = dense out: [h | alpha_src | alpha_dst]
NEG = 0.2
NCORES = 8
P = 128
NBLK = 20                 # dst blocks per core
NB_TOT = NCORES * NBLK    # 160
SLOTS = NBLK * P          # 2560 slots per core
TOT_SLOTS = SLOTS * NCORES
NQ = 4                    # AllGather quarters
NGQ = 4                   # SWDGE gather queues (Q7 pairs overlap across queues)
BPQ = NBLK // NQ          # local blocks per quarter (5)
QSH = SLOTS // NQ         # 640 rows per core per quarter
QTOT = TOT_SLOTS // NQ    # 5120 rows per quarter

F32 = mybir.dt.float32
F16 = mybir.dt.float16
I32 = mybir.dt.int32
I16 = mybir.dt.int16

EXPF = mybir.ActivationFunctionType.Exp
LRELU = mybir.ActivationFunctionType.Lrelu
RELU = mybir.ActivationFunctionType.Relu
COPYF = mybir.ActivationFunctionType.Copy


def _split_multiwaits(bir: bytes) -> bytes:
    """Walrus here allows only 1 sync wait per instruction -> hoist extras onto
    same-engine EventSemaphore waits (dedup repeated ge-waits per engine; sems
    are monotonic within the block, so a repeated >= wait is a no-op)."""
    j = orjson.loads(bir)
    ctr = 0
    for fn in j["functions"]:
        for blk in fn["blocks"]:
            out_l = []
            last_wait = {}
            for ins in blk["instructions"]:
                eng = ins.get("engine")
                si = ins.get("sync_info")
                ow = (si or {}).get("on_wait") or []
                keep = 1
                if len(ow) > keep:
                    seen = last_wait.setdefault(eng, set())
                    for w in ow[:len(ow) - keep]:
                        key = (w.get("id"), w.get("wait_mode"), w.get("wait_value"))
                        if w.get("wait_mode") == "sem-ge-imm":
                            if key in seen:
                                continue
                            seen.add(key)
                        ctr += 1
                        out_l.append({
                            "engine": eng, "ins": [], "outs": [],
                            "name": f"mwsplit-{ctr}", "opcode": "EventSemaphore",
                            "sync_info": {"on_update": [], "on_wait": [w]},
                        })
                    si["on_wait"] = ow[len(ow) - keep:]
                out_l.append(ins)
            blk["instructions"] = out_l
    return orjson.dumps(j)


class BassOneWait(bass.Bass):
    def to_json_bytes(self):
        return _split_multiwaits(super().to_json_bytes())


# ---------------------------------------------------------------- host prep

def _preprocess(edge_index):
    """Assign nodes to degree-balanced blocks; build per-core edge arrays.

    Self-loops (the reference's added arange loops) are NOT in the edge lists;
    they are handled by the per-block self tile (tile 0). Blocks are balanced
    on non-self in-degree.
    """
    src = np.asarray(edge_index[0], dtype=np.int64)
    dst = np.asarray(edge_index[1], dtype=np.int64)
    deg = np.bincount(dst, minlength=N_NODES).astype(np.int64)

    order = np.argsort(-deg, kind="stable")
    blk_of = np.empty(N_NODES, np.int32)
    slot_of = np.empty(N_NODES, np.int32)
    heap = [(0, 0, b) for b in range(NB_TOT)]
    heapq.heapify(heap)
    cnt = np.zeros(NB_TOT, np.int32)
    load = np.zeros(NB_TOT, np.int64)
    for n in order:
        while True:
            l, _, b = heapq.heappop(heap)
            if cnt[b] < P:
                break
        blk_of[n] = b
        slot_of[n] = cnt[b]
        cnt[b] += 1
        load[b] += deg[n]
        if cnt[b] < P:
            heapq.heappush(heap, (load[b], cnt[b], b))

    E_max = int(load.max())
    T = int(np.ceil(E_max / P))          # gather tiles per block
    TT = T + 1                           # + self tile
    gslot = blk_of.astype(np.int64) * P + slot_of
    node_of_slot = np.full(NB_TOT * P, -1, np.int64)
    node_of_slot[gslot] = np.arange(N_NODES)
    # table row under the quarter-split AllGather layout
    core_of = blk_of.astype(np.int64) // NBLK
    lb_of = blk_of.astype(np.int64) % NBLK
    q_of = lb_of // BPQ
    growt = (q_of * QTOT + core_of * QSH + (lb_of % BPQ) * P
             + slot_of.astype(np.int64))

    # bucket edges by dst block, sorted by src table row for HBM locality
    eb = blk_of[dst]
    order_e = np.lexsort((growt[src], eb))
    src_s = src[order_e]
    dst_s = dst[order_e]
    eb_s = eb[order_e]
    starts = np.searchsorted(eb_s, np.arange(NB_TOT + 1))

    N = T * P
    srcg = np.zeros((NCORES, P, NBLK * T * 8), np.int16)
    s_mat = np.zeros((NCORES, NBLK * TT, P, P), np.float16)   # [tile, edge, slot]
    st_mat = np.zeros((NCORES, NBLK * TT, P, P), np.float16)  # [tile, slot, edge]
    for b in range(NB_TOT):
        c, lb = divmod(b, NBLK)
        e0, e1 = starts[b], starts[b + 1]
        k = e1 - e0
        assert k <= E_max
        # stream i holds table-row idx; pad [k, N) with row 0 (every lane must
        # be written — unwritten lanes carry stale SBUF that can be NaN, and
        # the aggregation matmul propagates 0*NaN into PSUM)
        col = np.zeros(N, np.int64)
        col[:k] = growt[src_s[e0:e1]]
        # the block's T tiles are gathered by NGQ sub-gathers on separate SWDGE
        # queues; each sub-stream is independently wrapped: element i of a
        # sub-stream lives at [i % 16, i // 16], replicated over 8 Q7 cores
        tqs = _tq(T)
        off = 0
        pos = lb * (N // 16)
        for q in range(NGQ):
            Nq = tqs[q] * P
            sub = col[off * P:off * P + Nq]
            srcg[c, :, pos:pos + Nq // 16] = np.tile(
                sub.reshape(Nq // 16, 16).T, (8, 1)).astype(np.int16)
            off += tqs[q]
            pos += Nq // 16
        # self tile (tile 0): slot p loops to itself for occupied slots
        occ = np.nonzero(node_of_slot[b * P:(b + 1) * P] >= 0)[0]
        s_mat[c, lb * TT, occ, occ] = 1.0
        st_mat[c, lb * TT, occ, occ] = 1.0
        # gather tiles 1..T: edge at stream i -> (tile 1 + i//128, lane i%128)
        sl = np.full(N, -1, np.int64)
        sl[:k] = slot_of[dst_s[e0:e1]]
        sl2 = sl.reshape(T, P)
        tt_, ee = np.nonzero(sl2 >= 0)
        s_mat[c, lb * TT + 1 + tt_, ee, sl2[tt_, ee]] = 1.0
        st_mat[c, lb * TT + 1 + tt_, sl2[tt_, ee], ee] = 1.0

    # node id for each table row (shared by all cores)
    node_of_row = np.full(TOT_SLOTS, -1, np.int64)
    growt_all = growt[np.arange(N_NODES)]
    node_of_row[growt_all] = np.arange(N_NODES)
    return T, E_max, node_of_slot, node_of_row, srcg, s_mat, st_mat


def _aug_weights(W, a_src, a_dst, heads, hid):
    """[W | ws | wd], f16: ws[:,h] = W[:,h*hid:(h+1)*hid] @ a_src[h]."""
    cin = W.shape[0]
    ws = np.zeros((cin, heads), np.float32)
    wd = np.zeros((cin, heads), np.float32)
    for h in range(heads):
        blk = W[:, h * hid:(h + 1) * hid]
        ws[:, h] = blk @ a_src[h]
        wd[:, h] = blk @ a_dst[h]
    return np.concatenate([W, ws, wd], axis=1).astype(np.float16)


# ---------------------------------------------------------------- device kernel

def _tq(T):
    """Tiles per gather queue: split T tiles as evenly as possible."""
    return [T // NGQ + (q < T % NGQ) for q in range(NGQ)]


def _build(T, E_max):
    TT = T + 1
    NTT = NBLK * TT
    nc = BassOneWait(num_swdge_queues=NGQ)
    dp = nc.declare_dram_parameter
    xTf_in = dp("xTf_in", [P, NB_TOT * P], F16, isOutput=False)   # full x, row order
    xTl_in = dp("xTl_in", [P, NBLK * P], F16, isOutput=False)     # my shard
    srcg_in = dp("srcg_in", [P, NBLK * T * 8], I16, isOutput=False)
    s_in = dp("s_in", [P, NTT * P], F16, isOutput=False)
    st_in = dp("st_in", [P, NTT * P], F16, isOutput=False)
    wa1_in = dp("wa1_in", [IN_DIM, WAUG], F16, isOutput=False)
    wa2_in = dp("wa2_in", [HC, WAUG], F16, isOutput=False)
    w3_in = dp("w3_in", [1, HC], F16, isOutput=False)
    sc3_in = dp("sc3_in", [1, 4], F32, isOutput=False)
    as1_in = dp("as1_in", [1, HC], F16, isOutput=False)
    as2_in = dp("as2_in", [1, HC], F16, isOutput=False)
    b1_in = dp("b1_in", [1, HC], F32, isOutput=False)
    b2_in = dp("b2_in", [1, HC], F32, isOutput=False)
    ident_in = dp("ident_in", [P, P], F16, isOutput=False)
    out_p = dp("out_p", [P, NBLK], F32, isOutput=True)

    tab_full1 = nc.dram_tensor("tab_full1", [TOT_SLOTS, HC], F16)
    tab_sh2 = nc.dram_tensor("tab_sh2", [SLOTS, HC], F16)
    tab_full2 = nc.dram_tensor("tab_full2", [TOT_SLOTS, HC], F16)
    tab3_sh = nc.dram_tensor("tab3_sh", [SLOTS, 128], F16)
    tab3_full = nc.dram_tensor("tab3_full", [TOT_SLOTS, 128], F16)

    groups = [list(range(NCORES))]

    with tile.TileContext(nc) as tc, ExitStack() as ctx:
        consts = ctx.enter_context(tc.tile_pool(name="consts", bufs=1))
        meta = ctx.enter_context(tc.tile_pool(name="meta", bufs=1))
        spool = ctx.enter_context(tc.tile_pool(name="spool", bufs=3))
        gpool = ctx.enter_context(tc.tile_pool(name="gpool", bufs=4))
        mpool = ctx.enter_context(tc.tile_pool(name="mpool", bufs=2))
        small = ctx.enter_context(tc.tile_pool(name="small", bufs=2))
        sttp = ctx.enter_context(tc.tile_pool(name="sttp", bufs=3))
        stgp = ctx.enter_context(tc.tile_pool(name="stgp", bufs=2))
        psd = ctx.enter_context(tc.tile_pool(name="psd", bufs=2, space="PSUM"))
        pse = ctx.enter_context(tc.tile_pool(name="pse", bufs=2, space="PSUM"))
        pst = ctx.enter_context(tc.tile_pool(name="pst", bufs=2, space="PSUM"))
        psa = ctx.enter_context(tc.tile_pool(name="psa", bufs=2, space="PSUM"))

        nc.gpsimd.load_library(mlp)
        tqs = _tq(T)
        nidx_regs = {n: nc.gpsimd.to_reg(n * P) for n in set(tqs)}

        # ---- constants / metadata
        ident16 = consts.tile([P, P], F16)
        nc.sync.dma_start(out=ident16, in_=ident_in[:])
        wa1 = consts.tile([P, WAUG], F16)
        nc.sync.dma_start(out=wa1, in_=wa1_in[:])
        wa2 = consts.tile([P, 2, WAUG], F16)
        nc.sync.dma_start(out=wa2, in_=wa2_in.rearrange("(j p) a -> p j a", p=P))

        def rep_load(name, src, n, dt):
            t = consts.tile([P, n], dt, tag=name)
            bc = bass.AP(tensor=src.tensor, offset=0, ap=[[0, P], [1, n]])
            nc.sync.dma_start(out=t, in_=bc)
            return t
        w3r = rep_load("w3r", w3_in[:], HC, F16)
        as1r = rep_load("as1r", as1_in[:], HC, F16)
        as2r = rep_load("as2r", as2_in[:], HC, F16)
        sc3 = rep_load("sc3", sc3_in[:], 4, F32)
        b1r = rep_load("b1r", b1_in[:], HC, F32)
        b2r = rep_load("b2r", b2_in[:], HC, F32)

        srcg = meta.tile([P, NBLK * T * 8], I16)
        nc.sync.dma_start(out=srcg, in_=srcg_in[:])
        xTf = meta.tile([P, NB_TOT, P], F16)
        xTf_v = xTf_in.rearrange("p (b n) -> p b n", n=P)
        for qq in range(8):
            w8 = NB_TOT // 8
            nc.sync.dma_start(out=xTf[:, qq * w8:(qq + 1) * w8, :],
                              in_=xTf_v[:, qq * w8:(qq + 1) * w8, :])
        hT = meta.tile([P, 2 * NBLK, P], F16)
        nc.sync.dma_start(out=hT[:, 0:NBLK, :],
                          in_=xTl_in.rearrange("p (b n) -> p b n", n=P))
        outsb = meta.tile([P, NBLK], F32)
        denseT = meta.tile([P, NBLK, HC], F16, tag="denseT")
        adl = meta.tile([P, NBLK, HEADS], F16, tag="adl")
        adl3 = meta.tile([P, NBLK, 1], F16, tag="adl3")
        h3loc = meta.tile([P, NBLK, 2], F16, tag="h3loc")

        # ---------------- layer 1 dense: full table locally (no AllGather)
        # local pass first for denseT/adl (program-uniform block indices)
        for b in range(NBLK):
            ps = psd.tile([P, WAUG], F32, tag="dense")
            nc.tensor.matmul(ps, hT[:, b, :], wa1, start=True, stop=True)
            nc.scalar.activation(out=denseT[:, b, :], in_=ps[:, 0:HC], func=COPYF)
            nc.scalar.activation(out=adl[:, b, :], in_=ps[:, ROWW:WAUG], func=COPYF)
        tfv1 = tab_full1.rearrange("(b p) a -> p b a", p=P)
        GST = NB_TOT // 16         # 10 blocks per staged write
        for gq in range(16):
            stage = stgp.tile([P, GST, HC], F16, tag="stage")
            for gg in range(GST):
                g = gq * GST + gg
                ps = psd.tile([P, WAUG], F32, tag="dense")
                nc.tensor.matmul(ps, xTf[:, g, :], wa1, start=True, stop=True)
                nc.scalar.activation(out=stage[:, gg, :], in_=ps[:, 0:HC],
                                     func=COPYF)
            nc.sync.dma_start(out=tfv1[:, gq * GST:(gq + 1) * GST, :], in_=stage)

        def dense_block2(b):
            """Layer-2 dense for local block b (input: hT cols 2b, 2b+1)."""
            ps = psd.tile([P, WAUG], F32, tag="dense")
            nc.tensor.matmul(ps, hT[:, 2 * b, :], wa2[:, 0, :],
                             start=True, stop=False)
            nc.tensor.matmul(ps, hT[:, 2 * b + 1, :], wa2[:, 1, :],
                             start=False, stop=True)
            nc.scalar.activation(out=denseT[:, b, :], in_=ps[:, 0:HC], func=COPYF)
            nc.sync.dma_start(
                out=tab_sh2.rearrange("(b p) a -> p b a", p=P)[:, b, :],
                in_=denseT[:, b, :])
            nc.scalar.activation(out=adl[:, b, :], in_=ps[:, ROWW:WAUG], func=COPYF)

        s_view = s_in.rearrange("p (n q) -> p n q", q=P)    # [P, NTT, P]
        st_view = st_in.rearrange("p (n q) -> p n q", q=P)

        def load_s(b):
            S = spool.tile([P, TT, P], F16, tag="S")
            nc.sync.dma_start(out=S, in_=s_view[:, b * TT:(b + 1) * TT, :])
            St = sttp.tile([P, TT, P], F16, tag="St")
            nc.sync.dma_start(out=St, in_=st_view[:, b * TT:(b + 1) * TT, :])
            return S, St

        def ag(tsh, tfull, q):
            nc.gpsimd.collective_compute(
                "AllGather", mybir.AluOpType.bypass, replica_groups=groups,
                ins=[tsh[q * QSH:(q + 1) * QSH]],
                outs=[tfull[q * QTOT:(q + 1) * QTOT]])

        def edge12(lidx, tab_full, asr, brow, after_block, hooks):
            iw = T * 8
            for b in range(NBLK):
                    S, St = load_s(b)
                    hg = gpool.tile([P, TT, HC], F16, tag="hg")
                    # self tile: the block's own dense rows
                    nc.scalar.activation(out=hg[:, 0, :],
                                         in_=denseT[:, b, :], func=COPYF)
                    off = 0
                    pos = b * iw
                    for q in range(NGQ):
                        Tq = tqs[q]
                        nc.gpsimd.dma_gather(
                            hg[:, 1 + off:1 + off + Tq, :], tab_full[:],
                            srcg[:, pos:pos + Tq * 8],
                            Tq * P, nidx_regs[Tq], HC,
                            single_packet=False, queue_num=q)
                        off += Tq
                        pos += Tq * 8
                    # per-edge alpha_src: head-wise dot of gathered h with a_src
                    tmp = mpool.tile([P, TT, HC], F16, tag="astmp")
                    asr_b = bass.AP(tensor=asr.tensor, offset=asr.offset,
                                    ap=[list(asr.ap[0]), [0, TT],
                                        [asr.ap[-1][0], HC]])
                    nc.vector.tensor_tensor(out=tmp, in0=hg, in1=asr_b,
                                            op=mybir.AluOpType.mult)
                    asx = small.tile([P, TT, HEADS], F32, tag="asx")
                    nc.vector.tensor_reduce(
                        out=asx,
                        in_=tmp.rearrange("p t (h k) -> p t h k", h=HEADS),
                        axis=mybir.AxisListType.X, op=mybir.AluOpType.add)
                    # per-edge alpha_dst: St_t @ block's alpha_dst column
                    adx = psa.tile([P, TT, HEADS], F32, tag="adx")
                    for t in range(TT):
                        nc.tensor.matmul(adx[:, t, :], St[:, t, :], adl[:, b, :],
                                         start=True, stop=True)
                    asum = small.tile([P, TT, HEADS], F32, tag="asum")
                    nc.vector.tensor_tensor(out=asum, in0=adx, in1=asx,
                                            op=mybir.AluOpType.add)
                    lk = small.tile([P, TT, HEADS], F32, tag="lk")
                    nc.vector.tensor_scalar_mul(lk, asum, NEG)
                    nc.vector.tensor_tensor(out=lk, in0=lk, in1=asum,
                                            op=mybir.AluOpType.max)
                    exf = small.tile([P, TT, HEADS], F16, tag="exf")
                    nc.scalar.activation(out=exf, in_=lk, func=EXPF)
                    m = mpool.tile([P, TT, ROWW], F16, tag="m")
                    ex_b = bass.AP(tensor=exf.tensor, offset=exf.offset,
                                   ap=[exf.ap[0], exf.ap[1], exf.ap[2], [0, HID]])
                    nc.vector.tensor_tensor(
                        out=m[:, :, 0:HC].rearrange("p t (h k) -> p t h k", h=HEADS),
                        in0=hg.rearrange("p t (h k) -> p t h k", h=HEADS),
                        in1=ex_b, op=mybir.AluOpType.mult)
                    nc.scalar.activation(out=m[:, :, HC:ROWW], in_=exf, func=COPYF)

                    ps = pse.tile([P, ROWW], F32, tag="agg")
                    for t in range(TT):
                        nc.tensor.matmul(ps, S[:, t, :], m[:, t, :],
                                         start=(t == 0), stop=(t == TT - 1))

                    den = small.tile([P, HEADS], F32, tag="den")
                    nc.scalar.activation(out=den, in_=ps[:, HC:ROWW], func=COPYF,
                                         bias=1e-30)
                    rec = small.tile([P, HEADS], F32, tag="rec")
                    nc.vector.reciprocal(out=rec, in_=den)
                    rec_b = bass.AP(tensor=rec.tensor, offset=rec.offset,
                                    ap=[rec.ap[0], rec.ap[1], [0, HID]])
                    hn = small.tile([P, HC], F32, tag="hn")
                    nc.vector.tensor_tensor(
                        out=hn.rearrange("p (h k) -> p h k", h=HEADS),
                        in0=ps[:, 0:HC].rearrange("p (h k) -> p h k", h=HEADS),
                        in1=rec_b, op=mybir.AluOpType.mult)
                    nc.vector.tensor_tensor(out=hn, in0=hn, in1=brow,
                                            op=mybir.AluOpType.add)
                    emin = small.tile([P, HC], F32, tag="emin")
                    nc.scalar.activation(out=emin, in_=hn, func=RELU, scale=-1.0)
                    eex = small.tile([P, HC], F32, tag="eex")
                    nc.scalar.activation(out=eex, in_=emin, func=EXPF, scale=-1.0)
                    hnp = small.tile([P, HC], F32, tag="hnp")
                    nc.scalar.activation(out=hnp, in_=hn, func=RELU)
                    nc.vector.tensor_tensor(out=hn, in0=hnp, in1=eex,
                                            op=mybir.AluOpType.add)
                    hn16 = small.tile([P, HC], F16, tag="hn16")
                    nc.vector.tensor_scalar_add(hn16, hn, -1.0)
                    after_block(b, hn16)
                    if b in hooks:
                        hooks[b]()

        # ---------------- layer 1 edge (+ layer 2 dense interleaved)
        def after1(b, hn16):
            tp = pst.tile([P, P], F16, tag="tr")
            nc.tensor.transpose(out=tp, in_=hn16[:, 0:P], identity=ident16)
            nc.scalar.activation(out=hT[:, 2 * b, :], in_=tp, func=COPYF)
            tp2 = pst.tile([P, P], F16, tag="tr")
            nc.tensor.transpose(out=tp2, in_=hn16[:, P:HC], identity=ident16)
            nc.scalar.activation(out=hT[:, 2 * b + 1, :], in_=tp2, func=COPYF)
            dense_block2(b)
        hooks1 = {BPQ * (q + 1) - 1: (lambda q=q: ag(tab_sh2, tab_full2, q))
                  for q in range(NQ - 1)}
        edge12(0, tab_full1, as1r, b1r, after1, hooks1)
        ag(tab_sh2, tab_full2, NQ - 1)

        # ---------------- layer 2 edge (+ layer 3 dense inline)
        def after2(b, hn16):
            t3 = small.tile([P, HC], F16, tag="t3")
            nc.vector.tensor_tensor(out=t3, in0=hn16, in1=w3r,
                                    op=mybir.AluOpType.mult)
            h3 = small.tile([P, 1], F32, tag="h3")
            nc.vector.tensor_reduce(out=h3, in_=t3, axis=mybir.AxisListType.X,
                                    op=mybir.AluOpType.add)
            nc.scalar.activation(out=h3loc[:, b, 0:1], in_=h3, func=COPYF)
            nc.vector.tensor_tensor(out=h3loc[:, b, 1:2], in0=h3, in1=sc3[:, 0:1],
                                    op=mybir.AluOpType.mult)
            nc.sync.dma_start(
                out=tab3_sh.rearrange("(b p) a -> p b a", p=P)[:, b, 0:2],
                in_=h3loc[:, b, :])
            nc.vector.tensor_tensor(out=adl3[:, b, :], in0=h3, in1=sc3[:, 1:2],
                                    op=mybir.AluOpType.mult)
        hooks2 = {BPQ * (q + 1) - 1: (lambda q=q: ag(tab3_sh, tab3_full, q))
                  for q in range(NQ - 1)}
        edge12(1, tab_full2, as2r, b2r, after2, hooks2)
        ag(tab3_sh, tab3_full, NQ - 1)

        # ---------------- layer 3 edge
        iw = T * 8
        for b in range(NBLK):
                S, St = load_s(b)
                g3 = gpool.tile([P, TT, 128], F16, tag="g3")
                nc.scalar.activation(out=g3[:, 0, 0:2], in_=h3loc[:, b, :],
                                     func=COPYF)
                off = 0
                pos = b * iw
                for q in range(NGQ):
                    Tq = tqs[q]
                    nc.gpsimd.dma_gather(
                        g3[:, 1 + off:1 + off + Tq, :], tab3_full[:],
                        srcg[:, pos:pos + Tq * 8],
                        Tq * P, nidx_regs[Tq], 128,
                        single_packet=False, queue_num=q)
                    off += Tq
                    pos += Tq * 8
                d3 = psa.tile([P, TT, HEADS], F32, tag="adx")
                for t in range(TT):
                    nc.tensor.matmul(d3[:, t, 0:1], St[:, t, :], adl3[:, b, :],
                                     start=True, stop=True)
                e3 = small.tile([P, TT, 1], F32, tag="e3")
                nc.vector.tensor_tensor(out=e3, in0=g3[:, :, 1:2],
                                        in1=d3[:, :, 0:1],
                                        op=mybir.AluOpType.add)
                lk3 = small.tile([P, TT, 1], F32, tag="lk3")
                nc.vector.tensor_scalar_mul(lk3, e3, NEG)
                nc.vector.tensor_tensor(out=lk3, in0=lk3, in1=e3,
                                        op=mybir.AluOpType.max)
                ex3 = small.tile([P, TT, 1], F32, tag="ex3")
                nc.scalar.activation(out=ex3, in_=lk3, func=EXPF)
                m3 = small.tile([P, TT, 2], F16, tag="m3")
                nc.vector.tensor_tensor(out=m3[:, :, 0:1], in0=ex3,
                                        in1=g3[:, :, 0:1],
                                        op=mybir.AluOpType.mult)
                nc.scalar.activation(out=m3[:, :, 1:2], in_=ex3, func=COPYF)
                ps3f = pse.tile([P, ROWW], F32, tag="agg")
                ps3 = ps3f[:, 0:2]
                for t in range(TT):
                    nc.tensor.matmul(ps3, S[:, t, :], m3[:, t, :],
                                     start=(t == 0), stop=(t == TT - 1))
                den3 = small.tile([P, 1], F32, tag="den3")
                nc.scalar.activation(out=den3, in_=ps3[:, 1:2], func=COPYF,
                                     bias=1e-30)
                rec3 = small.tile([P, 1], F32, tag="rec3")
                nc.vector.reciprocal(out=rec3, in_=den3)
                nc.vector.tensor_tensor(out=outsb[:, b:b + 1], in0=ps3[:, 0:1],
                                        in1=rec3, op=mybir.AluOpType.mult)
        nc.vector.tensor_tensor(out=outsb, in0=outsb,
                                in1=bass.AP(tensor=sc3.tensor,
                                            offset=sc3[:, 2:3].offset,
                                            ap=[list(sc3.ap[0]), [0, NBLK]]),
                                op=mybir.AluOpType.add)
        nc.sync.dma_start(out=out_p[:], in_=outsb)

    mybir.codegen_inst_isa_subclasses(nc)
    return nc


_CACHE = {}


def kernel(x, edge_index, W1, a_src1, a_dst1, b1, W2, a_src2, a_dst2, b2,
           W3, a_src3, a_dst3, b3):
    (T, E_max, node_of_slot, node_of_row, srcg, s_mat,
     st_mat) = _preprocess(np.asarray(edge_index))

    wa1 = _aug_weights(np.asarray(W1, np.float32), np.asarray(a_src1, np.float32),
                       np.asarray(a_dst1, np.float32), HEADS, HID)
    wa2 = _aug_weights(np.asarray(W2, np.float32), np.asarray(a_src2, np.float32),
                       np.asarray(a_dst2, np.float32), HEADS, HID)
    w3 = np.asarray(W3, np.float32).reshape(1, HC).astype(np.float16)
    sc3 = np.array([[float(np.asarray(a_src3).reshape(-1)[0]),
                     float(np.asarray(a_dst3).reshape(-1)[0]),
                     float(np.asarray(b3).reshape(-1)[0]), 0.0]], np.float32)
    as1f = np.asarray(a_src1, np.float32).reshape(1, HC).astype(np.float16)
    as2f = np.asarray(a_src2, np.float32).reshape(1, HC).astype(np.float16)
    b1r = np.asarray(b1, np.float32).reshape(1, HC)
    b2r = np.asarray(b2, np.float32).reshape(1, HC)

    x = np.asarray(x, np.float32)
    # full x in table-row order (shared by all cores)
    xf = np.zeros((TOT_SLOTS, IN_DIM), np.float32)
    validf = node_of_row >= 0
    xf[validf] = x[node_of_row[validf]]
    xTfull = xf.T.astype(np.float16).reshape(P, NB_TOT * P)

    in_maps = []
    for c in range(NCORES):
        sl = slice(c * SLOTS, (c + 1) * SLOTS)
        nos = node_of_slot[sl]
        xs = np.zeros((SLOTS, IN_DIM), np.float32)
        valid = nos >= 0
        xs[valid] = x[nos[valid]]
        xTl = xs.T.astype(np.float16).reshape(P, SLOTS)
        in_maps.append({
            "xTf_in": xTfull,
            "xTl_in": xTl,
            "srcg_in": srcg[c],
            "s_in": np.ascontiguousarray(s_mat[c].transpose(1, 0, 2)).reshape(P, -1),
            "st_in": np.ascontiguousarray(st_mat[c].transpose(1, 0, 2)).reshape(P, -1),
            "wa1_in": wa1, "wa2_in": wa2, "w3_in": w3, "sc3_in": sc3,
            "as1_in": as1f, "as2_in": as2f,
            "b1_in": b1r, "b2_in": b2r,
            "ident_in": np.eye(P, dtype=np.float16),
        })

    key = (T, E_max)
    if key not in _CACHE:
        _CACHE[key] = _build(T, E_max)
    nc = _CACHE[key]
    res = run_bass_kernel_spmd(nc, in_maps, list(range(NCORES)))

    out = np.empty(N_NODES, np.float32)
    for c in range(NCORES):
        o = res.results[c]["out_p"]
        flat = o.T.reshape(-1)
        nos = node_of_slot[c * SLOTS:(c + 1) * SLOTS]
        valid = nos >= 0
        out[nos[valid]] = flat[valid]
    return out


# revision 16
# speedup vs baseline: 1.0903x; 1.0903x over previous
"""3-layer GAT (PyG GATConv semantics) on 8 Trainium2 NeuronCores — v4.

Strategy (dst-sharded, per-tile SWDGE gathers, lean compute):
- Nodes assigned to 160 degree-balanced blocks of <=128 dst slots; 20/core.
- Edge phase per block: tile 0 carries the 128 added self-loops and is filled by
  an SBUF copy of the block's own dense output (no gather); tiles 1..T carry
  non-self edges delivered by ONE dma_gather (SWDGE) of [h | alpha_src] rows.
  Per-block index streams are padded to a uniform E_max with row-0 dummies and
  -1 beyond, so the Q7 descriptor loop does exactly E_max rows per block.
- Layer 1's table is computed redundantly by every core from the full x (PE is
  mostly idle), eliminating the serial layer-1 AllGather from the prologue. A
  second 20-block pass over the core's own x shard fills denseT/adl (SPMD
  program is shared, so local-block extraction must use program-uniform
  indices). Layers 2/3 tables are AllGathered in per-quarter chunks fired from
  mid-edge hooks of the previous layer, shrinking inter-layer bubbles.
- The walrus in this toolchain accepts only ONE sync wait per instruction;
  BassOneWait splits Tile-generated multi-waits at serialization.
"""
import numpy as np
from contextlib import ExitStack
import heapq

import orjson
import concourse.bass as bass
import concourse.tile as tile
from concourse import mybir
from concourse.bass_utils import run_bass_kernel_spmd
from concourse.library_config import mlp

# problem constants (fixed by the harness's setup_inputs)
N_NODES = 20000
N_EDGES = 320000
IN_DIM = 128
HID = 64
HEADS = 4
HC = HEADS * HID          # 256
ROWW = HC + HEADS         # 260 = [h | alpha_src] (useful part)
ROWP = HC                 # table row: h only (512B, dma_gather 256B-aligned); per-edge
                          # alpha_src is recomputed on DVE from the gathered h
WAUG = HC + 2 * HEADS     # 264 = dense out: [h | alpha_src | alpha_dst]
NEG = 0.2
NCORES = 8
P = 128
NBLK = 20                 # dst blocks per core
NB_TOT = NCORES * NBLK    # 160
SLOTS = NBLK * P          # 2560 slots per core
TOT_SLOTS = SLOTS * NCORES
NGQ = 4                   # SWDGE gather queues (Q7 pairs overlap across queues)
# AllGather chunks (local blocks per chunk). The LAST chunk is a single block
# so the final inter-layer AllGather — the one that gates the next layer's
# first gather — is as small as possible.
AGCH = [5, 5, 5, 4, 1]
AGSTART = [0, 5, 10, 15, 19]          # first local block of each chunk
AGBASE = [0]                          # first table row of each chunk region
for _sz in AGCH[:-1]:
    AGBASE.append(AGBASE[-1] + NCORES * _sz * P)

F32 = mybir.dt.float32
F16 = mybir.dt.float16
I32 = mybir.dt.int32
I16 = mybir.dt.int16

EXPF = mybir.ActivationFunctionType.Exp
LRELU = mybir.ActivationFunctionType.Lrelu
RELU = mybir.ActivationFunctionType.Relu
COPYF = mybir.ActivationFunctionType.Copy


def _split_multiwaits(bir: bytes) -> bytes:
    """Walrus here allows only 1 sync wait per instruction -> hoist extras onto
    same-engine EventSemaphore waits (dedup repeated ge-waits per engine; sems
    are monotonic within the block, so a repeated >= wait is a no-op)."""
    j = orjson.loads(bir)
    ctr = 0
    for fn in j["functions"]:
        for blk in fn["blocks"]:
            out_l = []
            last_wait = {}
            for ins in blk["instructions"]:
                eng = ins.get("engine")
                si = ins.get("sync_info")
                ow = (si or {}).get("on_wait") or []
                keep = 1
                if len(ow) > keep:
                    seen = last_wait.setdefault(eng, set())
                    for w in ow[:len(ow) - keep]:
                        key = (w.get("id"), w.get("wait_mode"), w.get("wait_value"))
                        if w.get("wait_mode") == "sem-ge-imm":
                            if key in seen:
                                continue
                            seen.add(key)
                        ctr += 1
                        out_l.append({
                            "engine": eng, "ins": [], "outs": [],
                            "name": f"mwsplit-{ctr}", "opcode": "EventSemaphore",
                            "sync_info": {"on_update": [], "on_wait": [w]},
                        })
                    si["on_wait"] = ow[len(ow) - keep:]
                out_l.append(ins)
            blk["instructions"] = out_l
    return orjson.dumps(j)


class BassOneWait(bass.Bass):
    def to_json_bytes(self):
        return _split_multiwaits(super().to_json_bytes())


# ---------------------------------------------------------------- host prep

def _preprocess(edge_index):
    """Assign nodes to degree-balanced blocks; build per-core edge arrays.

    Self-loops (the reference's added arange loops) are NOT in the edge lists;
    they are handled by the per-block self tile (tile 0). Blocks are balanced
    on non-self in-degree.
    """
    src = np.asarray(edge_index[0], dtype=np.int64)
    dst = np.asarray(edge_index[1], dtype=np.int64)
    deg = np.bincount(dst, minlength=N_NODES).astype(np.int64)

    order = np.argsort(-deg, kind="stable")
    blk_of = np.empty(N_NODES, np.int32)
    slot_of = np.empty(N_NODES, np.int32)
    heap = [(0, 0, b) for b in range(NB_TOT)]
    heapq.heapify(heap)
    cnt = np.zeros(NB_TOT, np.int32)
    load = np.zeros(NB_TOT, np.int64)
    for n in order:
        while True:
            l, _, b = heapq.heappop(heap)
            if cnt[b] < P:
                break
        blk_of[n] = b
        slot_of[n] = cnt[b]
        cnt[b] += 1
        load[b] += deg[n]
        if cnt[b] < P:
            heapq.heappush(heap, (load[b], cnt[b], b))

    E_max = int(load.max())
    T = int(np.ceil(E_max / P))          # gather tiles per block
    TT = T + 1                           # + self tile
    gslot = blk_of.astype(np.int64) * P + slot_of
    node_of_slot = np.full(NB_TOT * P, -1, np.int64)
    node_of_slot[gslot] = np.arange(N_NODES)
    # table row under the chunk-split AllGather layout
    core_of = blk_of.astype(np.int64) // NBLK
    lb_of = blk_of.astype(np.int64) % NBLK
    ch_of = np.searchsorted(np.array(AGSTART), lb_of, side="right") - 1
    base = np.array(AGBASE, np.int64)[ch_of]
    szs = np.array(AGCH, np.int64)[ch_of]
    sts = np.array(AGSTART, np.int64)[ch_of]
    growt = (base + core_of * szs * P + (lb_of - sts) * P
             + slot_of.astype(np.int64))

    # bucket edges by dst block, sorted by src table row for HBM locality
    eb = blk_of[dst]
    order_e = np.lexsort((growt[src], eb))
    src_s = src[order_e]
    dst_s = dst[order_e]
    eb_s = eb[order_e]
    starts = np.searchsorted(eb_s, np.arange(NB_TOT + 1))

    N = T * P
    srcg = np.zeros((NCORES, P, NBLK * T * 8), np.int16)
    s_mat = np.zeros((NCORES, NBLK * TT, P, P), np.float16)   # [tile, edge, slot]
    st_mat = np.zeros((NCORES, NBLK * TT, P, P), np.float16)  # [tile, slot, edge]
    for b in range(NB_TOT):
        c, lb = divmod(b, NBLK)
        e0, e1 = starts[b], starts[b + 1]
        k = e1 - e0
        assert k <= E_max
        # stream i holds table-row idx; pad [k, N) with row 0 (every lane must
        # be written — unwritten lanes carry stale SBUF that can be NaN, and
        # the aggregation matmul propagates 0*NaN into PSUM)
        col = np.zeros(N, np.int64)
        col[:k] = growt[src_s[e0:e1]]
        # the block's T tiles are gathered by NGQ sub-gathers on separate SWDGE
        # queues; each sub-stream is independently wrapped: element i of a
        # sub-stream lives at [i % 16, i // 16], replicated over 8 Q7 cores
        tqs = _tq(T)
        off = 0
        pos = lb * (N // 16)
        for q in range(NGQ):
            Nq = tqs[q] * P
            sub = col[off * P:off * P + Nq]
            srcg[c, :, pos:pos + Nq // 16] = np.tile(
                sub.reshape(Nq // 16, 16).T, (8, 1)).astype(np.int16)
            off += tqs[q]
            pos += Nq // 16
        # self tile (tile 0): slot p loops to itself for occupied slots
        occ = np.nonzero(node_of_slot[b * P:(b + 1) * P] >= 0)[0]
        s_mat[c, lb * TT, occ, occ] = 1.0
        st_mat[c, lb * TT, occ, occ] = 1.0
        # gather tiles 1..T: edge at stream i -> (tile 1 + i//128, lane i%128)
        sl = np.full(N, -1, np.int64)
        sl[:k] = slot_of[dst_s[e0:e1]]
        sl2 = sl.reshape(T, P)
        tt_, ee = np.nonzero(sl2 >= 0)
        s_mat[c, lb * TT + 1 + tt_, ee, sl2[tt_, ee]] = 1.0
        st_mat[c, lb * TT + 1 + tt_, sl2[tt_, ee], ee] = 1.0

    # node id for each table row (shared by all cores)
    node_of_row = np.full(TOT_SLOTS, -1, np.int64)
    growt_all = growt[np.arange(N_NODES)]
    node_of_row[growt_all] = np.arange(N_NODES)
    return T, E_max, node_of_slot, node_of_row, srcg, s_mat, st_mat


def _aug_weights(W, a_src, a_dst, heads, hid):
    """[W | ws | wd], f16: ws[:,h] = W[:,h*hid:(h+1)*hid] @ a_src[h]."""
    cin = W.shape[0]
    ws = np.zeros((cin, heads), np.float32)
    wd = np.zeros((cin, heads), np.float32)
    for h in range(heads):
        blk = W[:, h * hid:(h + 1) * hid]
        ws[:, h] = blk @ a_src[h]
        wd[:, h] = blk @ a_dst[h]
    return np.concatenate([W, ws, wd], axis=1).astype(np.float16)


# ---------------------------------------------------------------- device kernel

def _tq(T):
    """Tiles per gather queue: split T tiles as evenly as possible."""
    return [T // NGQ + (q < T % NGQ) for q in range(NGQ)]


def _build(T, E_max, skip_bias):
    TT = T + 1
    NTT = NBLK * TT
    nc = BassOneWait(num_swdge_queues=NGQ)
    dp = nc.declare_dram_parameter
    xTf_in = dp("xTf_in", [P, NB_TOT * P], F16, isOutput=False)   # full x, row order
    xTl_in = dp("xTl_in", [P, NBLK * P], F16, isOutput=False)     # my shard
    srcg_in = dp("srcg_in", [P, NBLK * T * 8], I16, isOutput=False)
    s_in = dp("s_in", [P, NTT * P], F16, isOutput=False)
    st_in = dp("st_in", [P, NTT * P], F16, isOutput=False)
    wa1_in = dp("wa1_in", [IN_DIM, WAUG], F16, isOutput=False)
    wa2_in = dp("wa2_in", [HC, WAUG], F16, isOutput=False)
    w3_in = dp("w3_in", [1, HC], F16, isOutput=False)
    sc3_in = dp("sc3_in", [1, 4], F32, isOutput=False)
    as1_in = dp("as1_in", [1, HC], F16, isOutput=False)
    as2_in = dp("as2_in", [1, HC], F16, isOutput=False)
    b1_in = dp("b1_in", [1, HC], F32, isOutput=False)
    b2_in = dp("b2_in", [1, HC], F32, isOutput=False)
    ident_in = dp("ident_in", [P, P], F16, isOutput=False)
    out_p = dp("out_p", [P, NBLK], F32, isOutput=True)

    tab_full1 = nc.dram_tensor("tab_full1", [TOT_SLOTS, HC], F16)
    tab_sh2 = nc.dram_tensor("tab_sh2", [SLOTS, HC], F16)
    tab_full2 = nc.dram_tensor("tab_full2", [TOT_SLOTS, HC], F16)
    tab3_sh = nc.dram_tensor("tab3_sh", [SLOTS, 128], F16)
    tab3_full = nc.dram_tensor("tab3_full", [TOT_SLOTS, 128], F16)

    groups = [list(range(NCORES))]

    with tile.TileContext(nc) as tc, ExitStack() as ctx:
        consts = ctx.enter_context(tc.tile_pool(name="consts", bufs=1))
        meta = ctx.enter_context(tc.tile_pool(name="meta", bufs=1))
        spool = ctx.enter_context(tc.tile_pool(name="spool", bufs=3))
        gpool = ctx.enter_context(tc.tile_pool(name="gpool", bufs=5))
        mpool = ctx.enter_context(tc.tile_pool(name="mpool", bufs=2))
        small = ctx.enter_context(tc.tile_pool(name="small", bufs=2))
        sttp = ctx.enter_context(tc.tile_pool(name="sttp", bufs=3))
        stgp = ctx.enter_context(tc.tile_pool(name="stgp", bufs=2))
        psd = ctx.enter_context(tc.tile_pool(name="psd", bufs=2, space="PSUM"))
        pse = ctx.enter_context(tc.tile_pool(name="pse", bufs=2, space="PSUM"))
        pst = ctx.enter_context(tc.tile_pool(name="pst", bufs=2, space="PSUM"))
        psa = ctx.enter_context(tc.tile_pool(name="psa", bufs=2, space="PSUM"))

        nc.gpsimd.load_library(mlp)
        tqs = _tq(T)
        nidx_regs = {n: nc.gpsimd.to_reg(n * P) for n in set(tqs)}

        # ---- constants / metadata
        ident16 = consts.tile([P, P], F16)
        nc.sync.dma_start(out=ident16, in_=ident_in[:])
        wa1 = consts.tile([P, WAUG], F16)
        nc.sync.dma_start(out=wa1, in_=wa1_in[:])
        wa2 = consts.tile([P, 2, WAUG], F16)
        nc.sync.dma_start(out=wa2, in_=wa2_in.rearrange("(j p) a -> p j a", p=P))

        def rep_load(name, src, n, dt):
            t = consts.tile([P, n], dt, tag=name)
            bc = bass.AP(tensor=src.tensor, offset=0, ap=[[0, P], [1, n]])
            nc.sync.dma_start(out=t, in_=bc)
            return t
        w3r = rep_load("w3r", w3_in[:], HC, F16)
        as1r = rep_load("as1r", as1_in[:], HC, F16)
        as2r = rep_load("as2r", as2_in[:], HC, F16)
        sc3 = rep_load("sc3", sc3_in[:], 4, F32)
        b1r = rep_load("b1r", b1_in[:], HC, F32)
        b2r = rep_load("b2r", b2_in[:], HC, F32)

        srcg = meta.tile([P, NBLK * T * 8], I16)
        nc.sync.dma_start(out=srcg, in_=srcg_in[:])
        xTf = meta.tile([P, NB_TOT, P], F16)
        xTf_v = xTf_in.rearrange("p (b n) -> p b n", n=P)
        for qq in range(8):
            w8 = NB_TOT // 8
            nc.sync.dma_start(out=xTf[:, qq * w8:(qq + 1) * w8, :],
                              in_=xTf_v[:, qq * w8:(qq + 1) * w8, :])
        hT = meta.tile([P, 2 * NBLK, P], F16)
        nc.sync.dma_start(out=hT[:, 0:NBLK, :],
                          in_=xTl_in.rearrange("p (b n) -> p b n", n=P))
        outsb = meta.tile([P, NBLK], F32)
        denseT = meta.tile([P, NBLK, HC], F16, tag="denseT")
        adl = meta.tile([P, NBLK, HEADS], F16, tag="adl")
        adl3 = meta.tile([P, NBLK, 1], F16, tag="adl3")
        h3loc = meta.tile([P, NBLK, 2], F16, tag="h3loc")

        # ---------------- layer 1 dense: full table locally (no AllGather)
        # local pass first for denseT/adl (program-uniform block indices)
        for b in range(NBLK):
            ps = psd.tile([P, WAUG], F32, tag="dense")
            nc.tensor.matmul(ps, hT[:, b, :], wa1, start=True, stop=True)
            nc.scalar.activation(out=denseT[:, b, :], in_=ps[:, 0:HC], func=COPYF)
            nc.scalar.activation(out=adl[:, b, :], in_=ps[:, ROWW:WAUG], func=COPYF)
        tfv1 = tab_full1.rearrange("(b p) a -> p b a", p=P)
        GST = NB_TOT // 16         # 10 blocks per staged write
        for gq in range(16):
            stage = stgp.tile([P, GST, HC], F16, tag="stage")
            for gg in range(GST):
                g = gq * GST + gg
                ps = psd.tile([P, WAUG], F32, tag="dense")
                nc.tensor.matmul(ps, xTf[:, g, :], wa1, start=True, stop=True)
                if g % 2 == 0:
                    nc.scalar.activation(out=stage[:, gg, :], in_=ps[:, 0:HC],
                                         func=COPYF)
                else:
                    nc.vector.tensor_copy(stage[:, gg, :], ps[:, 0:HC])
            nc.sync.dma_start(out=tfv1[:, gq * GST:(gq + 1) * GST, :], in_=stage)

        def dense_block2(b):
            """Layer-2 dense for local block b (input: hT cols 2b, 2b+1)."""
            ps = psd.tile([P, WAUG], F32, tag="dense")
            nc.tensor.matmul(ps, hT[:, 2 * b, :], wa2[:, 0, :],
                             start=True, stop=False)
            nc.tensor.matmul(ps, hT[:, 2 * b + 1, :], wa2[:, 1, :],
                             start=False, stop=True)
            nc.scalar.activation(out=denseT[:, b, :], in_=ps[:, 0:HC], func=COPYF)
            nc.sync.dma_start(
                out=tab_sh2.rearrange("(b p) a -> p b a", p=P)[:, b, :],
                in_=denseT[:, b, :])
            nc.scalar.activation(out=adl[:, b, :], in_=ps[:, ROWW:WAUG], func=COPYF)

        s_view = s_in.rearrange("p (n q) -> p n q", q=P)    # [P, NTT, P]
        st_view = st_in.rearrange("p (n q) -> p n q", q=P)

        def load_s(b):
            S = spool.tile([P, TT, P], F16, tag="S")
            nc.sync.dma_start(out=S, in_=s_view[:, b * TT:(b + 1) * TT, :])
            St = sttp.tile([P, TT, P], F16, tag="St")
            nc.sync.dma_start(out=St, in_=st_view[:, b * TT:(b + 1) * TT, :])
            return S, St

        def ag(tsh, tfull, i):
            r0 = AGSTART[i] * P
            rows = AGCH[i] * P
            nc.gpsimd.collective_compute(
                "AllGather", mybir.AluOpType.bypass, replica_groups=groups,
                ins=[tsh[r0:r0 + rows]],
                outs=[tfull[AGBASE[i]:AGBASE[i] + NCORES * rows]])

        def edge12(lidx, tab_full, asr, brow, after_block, hooks):
            iw = T * 8
            for b in range(NBLK):
                    S, St = load_s(b)
                    hg = gpool.tile([P, TT, HC], F16, tag="hg")
                    # self tile: the block's own dense rows
                    nc.scalar.activation(out=hg[:, 0, :],
                                         in_=denseT[:, b, :], func=COPYF)
                    off = 0
                    pos = b * iw
                    for q in range(NGQ):
                        Tq = tqs[q]
                        nc.gpsimd.dma_gather(
                            hg[:, 1 + off:1 + off + Tq, :], tab_full[:],
                            srcg[:, pos:pos + Tq * 8],
                            Tq * P, nidx_regs[Tq], HC,
                            single_packet=False, queue_num=q)
                        off += Tq
                        pos += Tq * 8
                    # per-edge alpha_src: head-wise dot of gathered h with
                    # a_src, reduced straight into the PSUM tile the alpha_dst
                    # matmuls then accumulate onto
                    tmp = mpool.tile([P, TT, HC], F16, tag="astmp")
                    asr_b = bass.AP(tensor=asr.tensor, offset=asr.offset,
                                    ap=[list(asr.ap[0]), [0, TT],
                                        [asr.ap[-1][0], HC]])
                    nc.vector.tensor_tensor(out=tmp, in0=hg, in1=asr_b,
                                            op=mybir.AluOpType.mult)
                    asx = small.tile([P, TT, HEADS], F32, tag="asx")
                    nc.vector.tensor_reduce(
                        out=asx,
                        in_=tmp.rearrange("p t (h k) -> p t h k", h=HEADS),
                        axis=mybir.AxisListType.X, op=mybir.AluOpType.add)
                    # per-edge alpha_dst: St_t @ block's alpha_dst column
                    adx = psa.tile([P, TT, HEADS], F32, tag="adx")
                    for t in range(TT):
                        nc.tensor.matmul(adx[:, t, :], St[:, t, :], adl[:, b, :],
                                         start=True, stop=True)
                    asum = small.tile([P, TT, HEADS], F32, tag="asum")
                    nc.vector.tensor_tensor(out=asum, in0=adx, in1=asx,
                                            op=mybir.AluOpType.add)
                    lk = small.tile([P, TT, HEADS], F32, tag="lk")
                    nc.scalar.activation(out=lk, in_=asum, func=LRELU, alpha=NEG)
                    exf = small.tile([P, TT, HEADS], F16, tag="exf")
                    nc.scalar.activation(out=exf, in_=lk, func=EXPF)
                    m = mpool.tile([P, TT, ROWW], F16, tag="m")
                    ex_b = bass.AP(tensor=exf.tensor, offset=exf.offset,
                                   ap=[exf.ap[0], exf.ap[1], exf.ap[2], [0, HID]])
                    nc.vector.tensor_tensor(
                        out=m[:, :, 0:HC].rearrange("p t (h k) -> p t h k", h=HEADS),
                        in0=hg.rearrange("p t (h k) -> p t h k", h=HEADS),
                        in1=ex_b, op=mybir.AluOpType.mult)
                    nc.scalar.activation(out=m[:, :, HC:ROWW], in_=exf, func=COPYF)

                    ps = pse.tile([P, ROWW], F32, tag="agg")
                    for t in range(TT):
                        nc.tensor.matmul(ps, S[:, t, :], m[:, t, :],
                                         start=(t == 0), stop=(t == TT - 1))

                    den = small.tile([P, HEADS], F32, tag="den")
                    nc.scalar.activation(out=den, in_=ps[:, HC:ROWW], func=COPYF,
                                         bias=1e-30)
                    rec = small.tile([P, HEADS], F32, tag="rec")
                    nc.vector.reciprocal(out=rec, in_=den)
                    rec_b = bass.AP(tensor=rec.tensor, offset=rec.offset,
                                    ap=[rec.ap[0], rec.ap[1], [0, HID]])
                    hn = small.tile([P, HC], F32, tag="hn")
                    nc.vector.tensor_tensor(
                        out=hn.rearrange("p (h k) -> p h k", h=HEADS),
                        in0=ps[:, 0:HC].rearrange("p (h k) -> p h k", h=HEADS),
                        in1=rec_b, op=mybir.AluOpType.mult)
                    if brow is not None:
                        nc.vector.tensor_tensor(out=hn, in0=hn, in1=brow,
                                                op=mybir.AluOpType.add)
                    emin = small.tile([P, HC], F32, tag="emin")
                    nc.scalar.activation(out=emin, in_=hn, func=RELU, scale=-1.0)
                    eex = small.tile([P, HC], F32, tag="eex")
                    nc.scalar.activation(out=eex, in_=emin, func=EXPF, scale=-1.0)
                    hnp = small.tile([P, HC], F32, tag="hnp")
                    nc.scalar.activation(out=hnp, in_=hn, func=RELU)
                    nc.vector.tensor_tensor(out=hn, in0=hnp, in1=eex,
                                            op=mybir.AluOpType.add)
                    hn16 = small.tile([P, HC], F16, tag="hn16")
                    nc.scalar.activation(out=hn16, in_=hn, func=COPYF, bias=-1.0)
                    after_block(b, hn16)
                    if b in hooks:
                        hooks[b]()

        # ---------------- layer 1 edge (+ layer 2 dense interleaved)
        def after1(b, hn16):
            tp = pst.tile([P, P], F16, tag="tr")
            nc.tensor.transpose(out=tp, in_=hn16[:, 0:P], identity=ident16)
            nc.scalar.activation(out=hT[:, 2 * b, :], in_=tp, func=COPYF)
            tp2 = pst.tile([P, P], F16, tag="tr")
            nc.tensor.transpose(out=tp2, in_=hn16[:, P:HC], identity=ident16)
            nc.scalar.activation(out=hT[:, 2 * b + 1, :], in_=tp2, func=COPYF)
            dense_block2(b)
        hooks1 = {AGSTART[i] + AGCH[i] - 1: (lambda i=i: ag(tab_sh2, tab_full2, i))
                  for i in range(len(AGCH) - 1)}
        edge12(0, tab_full1, as1r, None if skip_bias else b1r, after1, hooks1)
        ag(tab_sh2, tab_full2, len(AGCH) - 1)

        # ---------------- layer 2 edge (+ layer 3 dense inline)
        def after2(b, hn16):
            t3 = small.tile([P, HC], F16, tag="t3")
            nc.vector.tensor_tensor(out=t3, in0=hn16, in1=w3r,
                                    op=mybir.AluOpType.mult)
            h3 = small.tile([P, 1], F32, tag="h3")
            nc.vector.tensor_reduce(out=h3, in_=t3, axis=mybir.AxisListType.X,
                                    op=mybir.AluOpType.add)
            nc.scalar.activation(out=h3loc[:, b, 0:1], in_=h3, func=COPYF)
            nc.vector.tensor_tensor(out=h3loc[:, b, 1:2], in0=h3, in1=sc3[:, 0:1],
                                    op=mybir.AluOpType.mult)
            nc.sync.dma_start(
                out=tab3_sh.rearrange("(b p) a -> p b a", p=P)[:, b, 0:2],
                in_=h3loc[:, b, :])
            nc.vector.tensor_tensor(out=adl3[:, b, :], in0=h3, in1=sc3[:, 1:2],
                                    op=mybir.AluOpType.mult)
        hooks2 = {AGSTART[i] + AGCH[i] - 1: (lambda i=i: ag(tab3_sh, tab3_full, i))
                  for i in range(len(AGCH) - 1)}
        edge12(1, tab_full2, as2r, None if skip_bias else b2r, after2, hooks2)
        ag(tab3_sh, tab3_full, len(AGCH) - 1)

        # ---------------- layer 3 edge
        iw = T * 8
        for b in range(NBLK):
                S, St = load_s(b)
                g3 = gpool.tile([P, TT, 128], F16, tag="g3")
                nc.scalar.activation(out=g3[:, 0, 0:2], in_=h3loc[:, b, :],
                                     func=COPYF)
                off = 0
                pos = b * iw
                for q in range(NGQ):
                    Tq = tqs[q]
                    nc.gpsimd.dma_gather(
                        g3[:, 1 + off:1 + off + Tq, :], tab3_full[:],
                        srcg[:, pos:pos + Tq * 8],
                        Tq * P, nidx_regs[Tq], 128,
                        single_packet=False, queue_num=q)
                    off += Tq
                    pos += Tq * 8
                d3 = psa.tile([P, TT, HEADS], F32, tag="adx")
                for t in range(TT):
                    nc.tensor.matmul(d3[:, t, 0:1], St[:, t, :], adl3[:, b, :],
                                     start=True, stop=True)
                e3 = small.tile([P, TT, 1], F32, tag="e3")
                nc.vector.tensor_tensor(out=e3, in0=g3[:, :, 1:2],
                                        in1=d3[:, :, 0:1],
                                        op=mybir.AluOpType.add)
                lk3 = small.tile([P, TT, 1], F32, tag="lk3")
                nc.scalar.activation(out=lk3, in_=e3, func=LRELU,
                                     alpha=NEG)
                ex3 = small.tile([P, TT, 1], F32, tag="ex3")
                nc.scalar.activation(out=ex3, in_=lk3, func=EXPF)
                m3 = small.tile([P, TT, 2], F16, tag="m3")
                nc.vector.tensor_tensor(out=m3[:, :, 0:1], in0=ex3,
                                        in1=g3[:, :, 0:1],
                                        op=mybir.AluOpType.mult)
                nc.scalar.activation(out=m3[:, :, 1:2], in_=ex3, func=COPYF)
                ps3f = pse.tile([P, ROWW], F32, tag="agg")
                ps3 = ps3f[:, 0:2]
                for t in range(TT):
                    nc.tensor.matmul(ps3, S[:, t, :], m3[:, t, :],
                                     start=(t == 0), stop=(t == TT - 1))
                den3 = small.tile([P, 1], F32, tag="den3")
                nc.scalar.activation(out=den3, in_=ps3[:, 1:2], func=COPYF,
                                     bias=1e-30)
                rec3 = small.tile([P, 1], F32, tag="rec3")
                nc.vector.reciprocal(out=rec3, in_=den3)
                nc.vector.tensor_tensor(out=outsb[:, b:b + 1], in0=ps3[:, 0:1],
                                        in1=rec3, op=mybir.AluOpType.mult)
        nc.vector.tensor_tensor(out=outsb, in0=outsb,
                                in1=bass.AP(tensor=sc3.tensor,
                                            offset=sc3[:, 2:3].offset,
                                            ap=[list(sc3.ap[0]), [0, NBLK]]),
                                op=mybir.AluOpType.add)
        nc.sync.dma_start(out=out_p[:], in_=outsb)

    mybir.codegen_inst_isa_subclasses(nc)
    return nc


_CACHE = {}


def kernel(x, edge_index, W1, a_src1, a_dst1, b1, W2, a_src2, a_dst2, b2,
           W3, a_src3, a_dst3, b3):
    (T, E_max, node_of_slot, node_of_row, srcg, s_mat,
     st_mat) = _preprocess(np.asarray(edge_index))

    wa1 = _aug_weights(np.asarray(W1, np.float32), np.asarray(a_src1, np.float32),
                       np.asarray(a_dst1, np.float32), HEADS, HID)
    wa2 = _aug_weights(np.asarray(W2, np.float32), np.asarray(a_src2, np.float32),
                       np.asarray(a_dst2, np.float32), HEADS, HID)
    w3 = np.asarray(W3, np.float32).reshape(1, HC).astype(np.float16)
    sc3 = np.array([[float(np.asarray(a_src3).reshape(-1)[0]),
                     float(np.asarray(a_dst3).reshape(-1)[0]),
                     float(np.asarray(b3).reshape(-1)[0]), 0.0]], np.float32)
    as1f = np.asarray(a_src1, np.float32).reshape(1, HC).astype(np.float16)
    as2f = np.asarray(a_src2, np.float32).reshape(1, HC).astype(np.float16)
    b1r = np.asarray(b1, np.float32).reshape(1, HC)
    b2r = np.asarray(b2, np.float32).reshape(1, HC)

    x = np.asarray(x, np.float32)
    # full x in table-row order (shared by all cores)
    xf = np.zeros((TOT_SLOTS, IN_DIM), np.float32)
    validf = node_of_row >= 0
    xf[validf] = x[node_of_row[validf]]
    xTfull = xf.T.astype(np.float16).reshape(P, NB_TOT * P)

    in_maps = []
    for c in range(NCORES):
        sl = slice(c * SLOTS, (c + 1) * SLOTS)
        nos = node_of_slot[sl]
        xs = np.zeros((SLOTS, IN_DIM), np.float32)
        valid = nos >= 0
        xs[valid] = x[nos[valid]]
        xTl = xs.T.astype(np.float16).reshape(P, SLOTS)
        in_maps.append({
            "xTf_in": xTfull,
            "xTl_in": xTl,
            "srcg_in": srcg[c],
            "s_in": np.ascontiguousarray(s_mat[c].transpose(1, 0, 2)).reshape(P, -1),
            "st_in": np.ascontiguousarray(st_mat[c].transpose(1, 0, 2)).reshape(P, -1),
            "wa1_in": wa1, "wa2_in": wa2, "w3_in": w3, "sc3_in": sc3,
            "as1_in": as1f, "as2_in": as2f,
            "b1_in": b1r, "b2_in": b2r,
            "ident_in": np.eye(P, dtype=np.float16),
        })

    skip_bias = bool(np.all(b1r == 0.0) and np.all(b2r == 0.0))
    key = (T, E_max, skip_bias)
    if key not in _CACHE:
        _CACHE[key] = _build(T, E_max, skip_bias)
    nc = _CACHE[key]
    res = run_bass_kernel_spmd(nc, in_maps, list(range(NCORES)))

    out = np.empty(N_NODES, np.float32)
    for c in range(NCORES):
        o = res.results[c]["out_p"]
        flat = o.T.reshape(-1)
        nos = node_of_slot[c * SLOTS:(c + 1) * SLOTS]
        valid = nos >= 0
        out[nos[valid]] = flat[valid]
    return out
